# revision 31
# baseline (speedup 1.0000x reference)
"""Trainium2 Bass kernel for nn_ClockAwareGNN (segment_reduce).

Model (reference, fp32):
    gp   = segment_mean(x, batch) @ W_base + b_base            # [B, 1]
    h    = relu(clock @ W1 + b1) @ W2 + b2                     # [N, 16]
    cp   = segment_mean(h, batch)                              # [B, 16]
    out  = relu([gp | cp] @ W3 + b3) @ W4 + b4                 # [B, 1]

Everything after the segment reductions is affine in per-graph quantities, so
the heavy per-node work collapses to fused segment sums:
    Sx[g] = sum of x rows in graph g           (128 cols)
    Sr[g] = sum of r rows in graph g           (R cols)
where r is either the raw clock (R=1; exact when b1 == 0 and clock >= 0 since
relu(c*W1) == c*relu(W1) elementwise for c >= 0) or the host-computed
relu(clock @ W1 + b1) (R=16 fallback).  Graph node counts come from the same
host-side searchsorted that drives the shard layout.

Device strategy (per core, 8-way data-parallel by graph):
  - the whole payload ships as fp8e4m3 (1 B/element).  Quantization error on
    the final output is ~2.5e-3 rel (segment means average ~2000 nodes, which
    shrinks per-element fp8 noise by ~sqrt(2000)); the gate is 2e-2.
  - nodes arrive as 256-row DoubleRow tiles; batch ids are sorted so each
    128-row subtile touches <= 2 graphs inside one 32-graph "window".
  - DVE builds one-hot assign tiles [128 nodes, 32 graphs] for a whole
    super-tile in one is_equal op (broadcast AP vs an iota pattern).
  - PE accumulates assign.T @ payload into PSUM [128 graphs, C] fp32 with
    fp8 DoubleRow matmuls (256 nodes per instruction, 2 MACs/cell/cycle).
  - tiny vector-engine epilogue computes the folded per-graph MLP.
"""

import math
import sys
import types

import numpy as np
import ml_dtypes

import concourse.bass as bass
import concourse.bacc as bacc
import concourse.tile as tile
from concourse import mybir
from concourse.bass_utils import run_bass_kernel_spmd


def _ensure_axon_hooks():
    """bass_utils' trace path does `from antenv.axon_hooks import ...`;
    some agent images lack that submodule. Install it (with the real NTFF
    hook when available) so trace=True degrades gracefully instead of
    raising ModuleNotFoundError."""
    try:
        import antenv  # noqa: F401
        import antenv.axon_hooks  # noqa: F401
        return
    except ImportError:
        pass
    try:
        import antenv
    except ImportError:
        return
    mod = types.ModuleType("antenv.axon_hooks")
    state = {"hook": None}
    mod.set_axon_ntff_profile_hook = lambda h: state.__setitem__("hook", h)
    mod.get_axon_ntff_profile_hook = lambda: state["hook"]
    sys.modules["antenv.axon_hooks"] = mod
    antenv.axon_hooks = mod
    try:
        from trn_agent_boot.trn_boot import _ntff_profile_via_ctypes
        mod.set_axon_ntff_profile_hook(
            _ntff_profile_via_ctypes("/opt/axon/libaxon_pjrt.so"))
    except Exception:
        pass
    # the trace path also uploads the NEFF dir to a bucket; in zero-egress
    # containers that raises — fall back to the local path.
    try:
        import concourse.bass_utils as _bu
        _orig_upload = _bu.upload_artifacts

        def _safe_upload(tmpdir):
            try:
                return _orig_upload(tmpdir)
            except Exception:
                return str(tmpdir)

        _bu.upload_artifacts = _safe_upload
    except Exception:
        pass


_ensure_axon_hooks()

BF16 = ml_dtypes.bfloat16
F8 = ml_dtypes.float8_e4m3

N_CORES = 8
N_GRAPHS = 1024
D = 128                 # feature dim of x
GPC = N_GRAPHS // N_CORES   # graphs per core = 128
W = 16                  # one-hot window width (one PSUM bank per window)
WPC = GPC // W          # windows per core = 8


def _build_program(S, ST, C):
    """Build the SPMD Bass/Tile program. Shapes are static; per-core data
    differences live entirely in the input tensors.

    S:  number of super-tiles
    ST: DoubleRow-tiles (256 nodes each) per super-tile
    C:  fp8 payload column count = 128 + R
    """
    fp32 = mybir.dt.float32
    bf16 = mybir.dt.bfloat16
    f8 = mybir.dt.float8e4
    n_dr = S * ST               # DoubleRow tiles per core
    n128 = 2 * n_dr             # 128-node subtiles per core
    T_w = n128 // WPC           # 128-node tiles per window
    half = T_w // 2             # DoubleRow tiles per window
    R = C - D
    C2 = 2 * C

    nc = bacc.Bacc("TRN2", target_bir_lowering=False, debug=False,
                   num_devices=N_CORES)

    xcc = nc.dram_tensor("xcc", [S, 128, ST * C2], f8, kind="ExternalInput").ap()
    brs = nc.dram_tensor("brs", [128, n128], bf16, kind="ExternalInput").ap()
    iota_c = nc.dram_tensor("iota_c", [128, 2 * ST * W], bf16, kind="ExternalInput").ap()
    wbase_b = nc.dram_tensor("wbase_b", [128, D], fp32, kind="ExternalInput").ap()
    v1_b = nc.dram_tensor("v1_b", [128, 32], fp32, kind="ExternalInput").ap()
    m2_b = nc.dram_tensor("m2_b", [128, R * 32], fp32, kind="ExternalInput").ap()
    v0_b = nc.dram_tensor("v0_b", [128, 32], fp32, kind="ExternalInput").ap()
    w4_b = nc.dram_tensor("w4_b", [128, 32], fp32, kind="ExternalInput").ap()
    bb_t = nc.dram_tensor("bb_t", [128, 1], fp32, kind="ExternalInput").ap()
    b4_t = nc.dram_tensor("b4_t", [128, 1], fp32, kind="ExternalInput").ap()
    rec_b = nc.dram_tensor("rec_b", [128, WPC], fp32, kind="ExternalInput").ap()
    out_d = nc.dram_tensor("out", [128, 1], fp32, kind="ExternalOutput").ap()

    xbufs = max(3, min(8, (120 * 1024) // (ST * C2)))

    with tile.TileContext(nc) as tc:
        with (
            tc.tile_pool(name="consts", bufs=1) as cpool,
            tc.tile_pool(name="xin", bufs=xbufs) as xpool,
            tc.tile_pool(name="assign", bufs=3) as apool,
            tc.tile_pool(name="epi", bufs=1) as epool,
            tc.tile_pool(name="ps", bufs=1, space="PSUM") as ppool,
        ):
            # ---- head constants at the head of the SYNC ring; payload
            # super 0 goes on the scalar ring so neither waits on the other.
            # The epilogue-only constants are DMA'd after the loop is
            # emitted so they don't delay the first is_equal / matmul.
            brall = cpool.tile([128, n128], bf16, tag="brall")
            nc.sync.dma_start(brall[:], brs)
            iota_t = cpool.tile([128, 2 * ST * W], bf16, tag="iota")
            nc.sync.dma_start(iota_t[:], iota_c)
            wb_t = cpool.tile([128, D], fp32, tag="wb")
            v1_t = cpool.tile([128, 32], fp32, tag="v1")
            m2_t = cpool.tile([128, R * 32], fp32, tag="m2")
            v0_t = cpool.tile([128, 32], fp32, tag="v0")
            w4_t = cpool.tile([128, 32], fp32, tag="w4")
            bbt = cpool.tile([128, 1], fp32, tag="bb")
            b4t = cpool.tile([128, 1], fp32, tag="b4")
            rec_t = cpool.tile([128, WPC], fp32, tag="rec")

            # One PSUM bank per window.  DoubleRow weights occupy 2x32
            # physical PE columns, and walrus only accepts dst partition
            # base 0 for them (col_grp 0x3), so every window accumulates
            # into partitions [0:32] of its own bank.
            banks = [ppool.tile([128, 512], fp32, tag=f"acc{w}",
                                name=f"acc{w}")
                     for w in range(WPC)]

            # ---- PE warmup: dummy matmuls on zeros keep the PE busy from
            # t~=5us so the HAM clock-gate is released (1.2 -> 2.4 GHz)
            # before the first real matmul arrives.  Their garbage PSUM
            # writes are discarded by the real start=True matmuls.
            zw = cpool.tile([128, 2 * W], f8, tag="zw")
            nc.vector.memset(zw[:], 0.0)
            zr = cpool.tile([128, C2], f8, tag="zr")
            nc.vector.memset(zr[:], 0.0)
            for _ in range(16):
                nc.tensor.matmul(
                    banks[0][0:W, 0:C],
                    zw[:].rearrange("p (ko m) -> p ko m", ko=2),
                    zr[:].rearrange("p (ko c) -> p ko c", ko=2),
                    start=False, stop=False,
                    perf_mode=mybir.MatmulPerfMode.DoubleRow,
                    tile_position=(0, 0),
                    skip_group_check=True,
                )

            # ---- main loop ----
            for s in range(S):
                xt = xpool.tile([128, ST * C2], f8, tag="xt")
                # alternate the two HWDGE rings for the payload stream
                eng = nc.scalar if (s % 2 == 0) else nc.sync
                eng.dma_start(xt[:], xcc[s])
                # one-hot assign for all 2*ST subtiles in one op, alternating
                # DVE / GpSimd: asg[p, t, j] = (iota[j] == br[p, s*2*ST + t])
                veng = nc.vector
                asg = apool.tile([128, 2 * ST * W], f8, tag="asg")
                veng.tensor_tensor(
                    asg[:].rearrange("p (t j) -> p t j", j=W),
                    iota_t[:].rearrange("p (t j) -> p t j", j=W),
                    brall[:, s * 2 * ST : (s + 1) * 2 * ST]
                        .rearrange("p (t o) -> p t o", o=1)
                        .to_broadcast((128, 2 * ST, W)),
                    op=mybir.AluOpType.is_equal,
                )
                for t in range(ST):
                    i = s * ST + t          # DoubleRow tile index
                    w = (2 * i) // T_w      # window index (T_w is even)
                    iw = i - w * half       # index within window
                    nc.tensor.matmul(
                        banks[w][0:W, 0:C],
                        asg[:, t * 2 * W : (t + 1) * 2 * W]
                            .rearrange("p (ko m) -> p ko m", ko=2),
                        xt[:, t * C2 : (t + 1) * C2]
                            .rearrange("p (ko c) -> p ko c", ko=2),
                        start=(iw == 0),
                        stop=(iw == half - 1),
                        perf_mode=mybir.MatmulPerfMode.DoubleRow,
                        tile_position=(0, 0),
                    )

            # ---- epilogue-only constants, behind the payload stream ----
            nc.scalar.dma_start(wb_t[:], wbase_b)
            nc.scalar.dma_start(v1_t[:], v1_b)
            nc.scalar.dma_start(m2_t[:], m2_b)
            nc.scalar.dma_start(v0_t[:], v0_b)
            nc.scalar.dma_start(w4_t[:], w4_b)
            nc.scalar.dma_start(bbt[:], bb_t)
            nc.scalar.dma_start(b4t[:], b4_t)
            nc.scalar.dma_start(rec_t[:], rec_b)

            # ---- epilogue (per-graph folded MLP), one pass per window ----
            # window w lives in partitions [0:32] of banks[w]; outputs are
            # assembled as columns of one [32, WPC] tile.
            oo = epool.tile([128, WPC], fp32, tag="oo")
            for w in range(WPC):
                ew = nc.vector
                rw = rec_t[0:W, w : w + 1]

                # drain PSUM -> SBUF (GpSimd cannot read PSUM)
                sbw = epool.tile([128, C], fp32, tag=f"sb{w}")
                nc.vector.tensor_copy(sbw[0:W], banks[w][0:W, 0:C])

                # t1 = (Sx * recip) * W_base ; gp0 = rowsum(t1)
                t1 = epool.tile([128, D], fp32, tag=f"t1{w}")
                gp = epool.tile([128, 1], fp32, tag=f"gp{w}")
                ew.scalar_tensor_tensor(
                    t1[0:W], sbw[0:W, 0:D], rw, wb_t[0:W],
                    op0=mybir.AluOpType.mult, op1=mybir.AluOpType.mult,
                    accum_out=gp[0:W],
                )
                # mr = Sr * recip
                mr = epool.tile([128, R], fp32, tag=f"mr{w}")
                ew.tensor_scalar_mul(mr[0:W], sbw[0:W, D : D + R], rw)
                # gp += b_base
                ew.tensor_add(gp[0:W], gp[0:W], bbt[0:W])
                # pre = gp*v1 + v0  (+ sum_j mr[:,j]*M2[j])
                pre = epool.tile([128, 32], fp32, tag=f"pre{w}")
                ew.scalar_tensor_tensor(
                    pre[0:W], v1_t[0:W], gp[0:W], v0_t[0:W],
                    op0=mybir.AluOpType.mult, op1=mybir.AluOpType.add,
                )
                for j in range(R):
                    ew.scalar_tensor_tensor(
                        pre[0:W], m2_t[0:W, j * 32 : (j + 1) * 32],
                        mr[0:W, j : j + 1], pre[0:W],
                        op0=mybir.AluOpType.mult, op1=mybir.AluOpType.add,
                    )
                act = epool.tile([128, 32], fp32, tag=f"act{w}")
                nc.scalar.activation(act[0:W], pre[0:W],
                                     mybir.ActivationFunctionType.Relu)
                # oo[:, w] = rowsum(act * W4) + b4
                prod = epool.tile([128, 32], fp32, tag=f"prod{w}")
                ew.tensor_mul(prod[0:W], act[0:W], w4_t[0:W])
                ew.tensor_reduce(oo[0:W, w : w + 1], prod[0:W],
                                 axis=mybir.AxisListType.X,
                                 op=mybir.AluOpType.add)
                ew.tensor_add(oo[0:W, w : w + 1], oo[0:W, w : w + 1], b4t[0:W])

            # out row w*32+p  <-  oo[p, w]
            nc.sync.dma_start(
                out_d.rearrange("(w p) o -> p (w o)", w=WPC), oo[0:W, :])

    nc.compile()
    return nc


def kernel(x, clock_period, batch, W_base, b_base, W1, b1, W2, b2, W3, b3, W4, b4,
           _profile=None):
    x = np.asarray(x, np.float32)
    clock = np.asarray(clock_period, np.float32).reshape(-1)
    batch = np.asarray(batch, np.int32)
    W_base = np.asarray(W_base, np.float32)
    W1 = np.asarray(W1, np.float32); b1 = np.asarray(b1, np.float32)
    W2 = np.asarray(W2, np.float32); b2 = np.asarray(b2, np.float32)
    W3 = np.asarray(W3, np.float32); b3 = np.asarray(b3, np.float32)
    W4 = np.asarray(W4, np.float32); b4 = np.asarray(b4, np.float32)
    hid = W1.shape[1]

    # r-path: exact algebraic fold when relu(c*W1 + b1) == c * relu(W1)
    fold = bool(np.all(b1 == 0.0)) and bool(clock.min() >= 0.0)
    if fold:
        R = 1
        r32 = clock[:, None]                                   # [N, 1]
        q = np.maximum(W1, 0.0) @ W2                           # [1, hid]
        M2 = q @ W3[1:, :]                                     # [1, 32]
        v0 = b2 @ W3[1:, :] + b3                               # [32]
    else:
        R = hid
        r32 = np.maximum(clock[:, None] @ W1 + b1, 0.0)        # [N, hid]
        M2 = W2 @ W3[1:, :]                                    # [hid, 32]
        v0 = b2 @ W3[1:, :] + b3

    C = D + R               # fp8 payload: [x | r]
    C2 = 2 * C

    # ---- shard by graph; window padding so tile->window map is static ----
    cut = np.searchsorted(batch, np.arange(0, N_GRAPHS + 1, W))
    win_nodes = np.diff(cut)
    T_w = int(math.ceil(win_nodes.max() / 128.0))
    # T_w even (DoubleRow pairs stay in-window); pick a super-tile size
    # ST | n_dr with ST in [36, 52] (~1.2-1.7 MB DMA transfers)
    def _pick_st(n_dr):
        for st in range(52, 35, -1):
            if n_dr % st == 0:
                return st
        return None
    T_w += T_w % 2
    while True:
        n_dr = WPC * T_w // 2
        ST = _pick_st(n_dr)
        if ST is not None:
            break
        T_w += 2
    n128 = WPC * T_w
    S = n_dr // ST
    Npad = n128 * 128

    x8 = x.astype(F8)
    r8 = r32.astype(F8)

    # per-graph counts (same host metadata as the shard layout)
    counts = np.diff(np.searchsorted(batch, np.arange(0, N_GRAPHS + 1)))
    recip = (1.0 / np.maximum(counts, 1)).astype(np.float32)

    in_maps = []
    # shared constant tiles
    iota_c = np.broadcast_to(
        np.tile(np.arange(W, dtype=BF16), 2 * ST)[None, :], (128, 2 * ST * W)
    ).copy()
    wbase_b = np.broadcast_to(W_base[:, 0][None, :], (128, D)).astype(np.float32).copy()
    v1_b = np.broadcast_to(W3[0, :][None, :], (128, 32)).astype(np.float32).copy()
    m2_b = np.broadcast_to(M2.reshape(-1)[None, :], (128, R * 32)).astype(np.float32).copy()
    v0_b = np.broadcast_to(v0[None, :], (128, 32)).astype(np.float32).copy()
    w4_b = np.broadcast_to(W4[:, 0][None, :], (128, 32)).astype(np.float32).copy()
    bb_t = np.full((128, 1), float(b_base.reshape(-1)[0]), np.float32)
    b4_t = np.full((128, 1), float(b4.reshape(-1)[0]), np.float32)

    for k in range(N_CORES):
        xcc = np.zeros((Npad, C), F8)
        br = np.full(Npad, -1.0, BF16)
        for wi in range(WPC):
            gw = k * WPC + wi          # global window index
            s0, e0 = int(cut[gw]), int(cut[gw + 1])
            n = e0 - s0
            o = wi * T_w * 128
            xcc[o : o + n, 0:D] = x8[s0:e0]
            xcc[o : o + n, D : D + R] = r8[s0:e0]
            br[o : o + n] = (batch[s0:e0] - gw * W).astype(BF16)
        brs = np.ascontiguousarray(br.reshape(n128, 128).T)
        # DoubleRow packing: DR-tile d holds nodes [d*256, (d+1)*256), with
        # slot (p, ko) = node d*256 + ko*128 + p; each partition line is
        # contiguous in DRAM per super-tile.
        xcc_p = np.ascontiguousarray(
            xcc.reshape(S, ST, 2, 128, C).transpose(0, 3, 1, 2, 4)
        ).reshape(S, 128, ST * C2)
        # col w, partitions [0:32] = window w's graphs
        rk = recip[k * GPC : (k + 1) * GPC]
        rec_k = np.ones((128, WPC), np.float32)
        rec_k[0:W, :] = rk.reshape(WPC, W).T
        in_maps.append(dict(
            xcc=xcc_p, brs=brs, iota_c=iota_c,
            wbase_b=wbase_b, v1_b=v1_b, m2_b=m2_b, v0_b=v0_b, w4_b=w4_b,
            bb_t=bb_t, b4_t=b4_t, rec_b=rec_k,
        ))

    nc = _build_program(S, ST, C)

    kw = {}
    if _profile is not None:
        kw = dict(trace=True, **_profile)
    res = run_bass_kernel_spmd(nc, in_maps, list(range(N_CORES)), **kw)

    out = np.concatenate([res.results[k]["out"] for k in range(N_CORES)], axis=0)
    if _profile is not None:
        return out.astype(np.float32), res
    return out.astype(np.float32)


# revision 37
# speedup vs baseline: 1.0266x; 1.0266x over previous
"""Trainium2 Bass kernel for nn_ClockAwareGNN (segment_reduce).

Model (reference, fp32):
    gp   = segment_mean(x, batch) @ W_base + b_base            # [B, 1]
    h    = relu(clock @ W1 + b1) @ W2 + b2                     # [N, 16]
    cp   = segment_mean(h, batch)                              # [B, 16]
    out  = relu([gp | cp] @ W3 + b3) @ W4 + b4                 # [B, 1]

Everything after the segment reductions is affine in per-graph quantities, so
the heavy per-node work collapses to fused segment sums:
    Sx[g] = sum of x rows in graph g           (128 cols)
    Sr[g] = sum of r rows in graph g           (R cols)
where r is either the raw clock (R=1; exact when b1 == 0 and clock >= 0 since
relu(c*W1) == c*relu(W1) elementwise for c >= 0) or the host-computed
relu(clock @ W1 + b1) (R=16 fallback).  Graph node counts come from the same
host-side searchsorted that drives the shard layout.

Device strategy (per core, 8-way data-parallel by graph):
  - the whole payload ships as fp8e4m3 (1 B/element).  Quantization error on
    the final output is ~2.5e-3 rel (segment means average ~2000 nodes, which
    shrinks per-element fp8 noise by ~sqrt(2000)); the gate is 2e-2.
  - nodes arrive as 256-row DoubleRow tiles; batch ids are sorted so each
    128-row subtile touches <= 2 graphs inside one 32-graph "window".
  - DVE builds one-hot assign tiles [128 nodes, 32 graphs] for a whole
    super-tile in one is_equal op (broadcast AP vs an iota pattern).
  - PE accumulates assign.T @ payload into PSUM [128 graphs, C] fp32 with
    fp8 DoubleRow matmuls (256 nodes per instruction, 2 MACs/cell/cycle).
  - tiny vector-engine epilogue computes the folded per-graph MLP.
"""

import math
import sys
import types

import numpy as np
import ml_dtypes

import concourse.bass as bass
import concourse.bacc as bacc
import concourse.tile as tile
from concourse import mybir
from concourse.bass_utils import run_bass_kernel_spmd


def _ensure_axon_hooks():
    """bass_utils' trace path does `from antenv.axon_hooks import ...`;
    some agent images lack that submodule. Install it (with the real NTFF
    hook when available) so trace=True degrades gracefully instead of
    raising ModuleNotFoundError."""
    try:
        import antenv  # noqa: F401
        import antenv.axon_hooks  # noqa: F401
        return
    except ImportError:
        pass
    try:
        import antenv
    except ImportError:
        return
    mod = types.ModuleType("antenv.axon_hooks")
    state = {"hook": None}
    mod.set_axon_ntff_profile_hook = lambda h: state.__setitem__("hook", h)
    mod.get_axon_ntff_profile_hook = lambda: state["hook"]
    sys.modules["antenv.axon_hooks"] = mod
    antenv.axon_hooks = mod
    try:
        from trn_agent_boot.trn_boot import _ntff_profile_via_ctypes
        mod.set_axon_ntff_profile_hook(
            _ntff_profile_via_ctypes("/opt/axon/libaxon_pjrt.so"))
    except Exception:
        pass
    # the trace path also uploads the NEFF dir to a bucket; in zero-egress
    # containers that raises — fall back to the local path.
    try:
        import concourse.bass_utils as _bu
        _orig_upload = _bu.upload_artifacts

        def _safe_upload(tmpdir):
            try:
                return _orig_upload(tmpdir)
            except Exception:
                return str(tmpdir)

        _bu.upload_artifacts = _safe_upload
    except Exception:
        pass


_ensure_axon_hooks()

BF16 = ml_dtypes.bfloat16
F8 = ml_dtypes.float8_e4m3

N_CORES = 8
N_GRAPHS = 1024
D = 128                 # feature dim of x
GPC = N_GRAPHS // N_CORES   # graphs per core = 128
W = 16                  # one-hot window width (one PSUM bank per window)
WPC = GPC // W          # windows per core = 8


def _build_program(S, ST, C):
    """Build the SPMD Bass/Tile program. Shapes are static; per-core data
    differences live entirely in the input tensors.

    S:  number of super-tiles
    ST: DoubleRow-tiles (256 nodes each) per super-tile
    C:  fp8 payload column count = 128 + R
    """
    fp32 = mybir.dt.float32
    bf16 = mybir.dt.bfloat16
    f8 = mybir.dt.float8e4
    n_dr = S * ST               # DoubleRow tiles per core
    n128 = 2 * n_dr             # 128-node subtiles per core
    T_w = n128 // WPC           # 128-node tiles per window
    half = T_w // 2             # DoubleRow tiles per window
    R = C - D
    C2 = 2 * C

    nc = bacc.Bacc("TRN2", target_bir_lowering=False, debug=False,
                   num_devices=N_CORES)

    xcc = nc.dram_tensor("xcc", [S, 128, ST * C2], f8, kind="ExternalInput").ap()
    brs = nc.dram_tensor("brs", [128, n128], f8, kind="ExternalInput").ap()
    iota_c = nc.dram_tensor("iota_c", [128, 2 * ST * W], f8, kind="ExternalInput").ap()
    wbase_b = nc.dram_tensor("wbase_b", [128, D], fp32, kind="ExternalInput").ap()
    v1_b = nc.dram_tensor("v1_b", [128, 32], fp32, kind="ExternalInput").ap()
    m2_b = nc.dram_tensor("m2_b", [128, R * 32], fp32, kind="ExternalInput").ap()
    v0_b = nc.dram_tensor("v0_b", [128, 32], fp32, kind="ExternalInput").ap()
    w4_b = nc.dram_tensor("w4_b", [128, 32], fp32, kind="ExternalInput").ap()
    bb_t = nc.dram_tensor("bb_t", [128, 1], fp32, kind="ExternalInput").ap()
    b4_t = nc.dram_tensor("b4_t", [128, 1], fp32, kind="ExternalInput").ap()
    rec_b = nc.dram_tensor("rec_b", [128, WPC], fp32, kind="ExternalInput").ap()
    out_d = nc.dram_tensor("out", [128, 1], fp32, kind="ExternalOutput").ap()

    xbufs = max(3, min(8, (120 * 1024) // (ST * C2)))

    with tile.TileContext(nc) as tc:
        with (
            tc.tile_pool(name="consts", bufs=1) as cpool,
            tc.tile_pool(name="xin", bufs=xbufs) as xpool,
            tc.tile_pool(name="assign", bufs=3) as apool,
            tc.tile_pool(name="epi", bufs=1) as epool,
            tc.tile_pool(name="ps", bufs=1, space="PSUM") as ppool,
        ):
            # ---- head constants at the head of the SYNC ring; payload
            # super 0 goes on the scalar ring so neither waits on the other.
            # Super 0's batch-rel ids come in a tiny separate DMA so the
            # first is_equal unblocks in ~1us instead of after the whole
            # brall transfer.  The epilogue-only constants are DMA'd after
            # the loop is emitted so they don't delay the payload stream.
            iota_t = cpool.tile([128, 2 * ST * W], f8, tag="iota")
            nc.sync.dma_start(iota_t[:], iota_c)
            br0 = cpool.tile([128, 2 * ST], f8, tag="br0")
            nc.sync.dma_start(br0[:], brs[:, 0 : 2 * ST])
            brall = cpool.tile([128, n128], f8, tag="brall")
            nc.sync.dma_start(brall[:, 2 * ST :], brs[:, 2 * ST :])
            wb_t = cpool.tile([128, D], fp32, tag="wb")
            v1_t = cpool.tile([128, 32], fp32, tag="v1")
            m2_t = cpool.tile([128, R * 32], fp32, tag="m2")
            v0_t = cpool.tile([128, 32], fp32, tag="v0")
            w4_t = cpool.tile([128, 32], fp32, tag="w4")
            bbt = cpool.tile([128, 1], fp32, tag="bb")
            b4t = cpool.tile([128, 1], fp32, tag="b4")
            rec_t = cpool.tile([128, WPC], fp32, tag="rec")

            # One PSUM bank per window.  DoubleRow weights occupy 2x32
            # physical PE columns, and walrus only accepts dst partition
            # base 0 for them (col_grp 0x3), so every window accumulates
            # into partitions [0:32] of its own bank.
            banks = [ppool.tile([128, 512], fp32, tag=f"acc{w}",
                                name=f"acc{w}")
                     for w in range(WPC)]

            # ---- PE warmup: dummy matmuls on zeros keep the PE busy from
            # t~=5us so the HAM clock-gate is released (1.2 -> 2.4 GHz)
            # before the first real matmul arrives.  Their garbage PSUM
            # writes are discarded by the real start=True matmuls.
            zw = cpool.tile([128, 2 * W], f8, tag="zw")
            nc.vector.memset(zw[:], 0.0)
            zr = cpool.tile([128, C2], f8, tag="zr")
            nc.vector.memset(zr[:], 0.0)
            for _ in range(16):
                nc.tensor.matmul(
                    banks[0][0:W, 0:C],
                    zw[:].rearrange("p (ko m) -> p ko m", ko=2),
                    zr[:].rearrange("p (ko c) -> p ko c", ko=2),
                    start=False, stop=False,
                    perf_mode=mybir.MatmulPerfMode.DoubleRow,
                    tile_position=(0, 0),
                    skip_group_check=True,
                )

            # ---- main loop ----
            for s in range(S):
                xt = xpool.tile([128, ST * C2], f8, tag="xt")
                # alternate the two HWDGE rings for the payload stream
                eng = nc.scalar if (s % 2 == 0) else nc.sync
                eng.dma_start(xt[:], xcc[s])
                # one-hot assign for all 2*ST subtiles in one op, alternating
                # DVE / GpSimd: asg[p, t, j] = (iota[j] == br[p, s*2*ST + t])
                br_src = br0[:] if s == 0 else \
                    brall[:, s * 2 * ST : (s + 1) * 2 * ST]
                asg = apool.tile([128, 2 * ST * W], f8, tag="asg")
                nc.vector.tensor_tensor(
                    asg[:].rearrange("p (t j) -> p t j", j=W),
                    iota_t[:].rearrange("p (t j) -> p t j", j=W),
                    br_src.rearrange("p (t o) -> p t o", o=1)
                        .to_broadcast((128, 2 * ST, W)),
                    op=mybir.AluOpType.is_equal,
                )
                for t in range(ST):
                    i = s * ST + t          # DoubleRow tile index
                    w = (2 * i) // T_w      # window index (T_w is even)
                    iw = i - w * half       # index within window
                    nc.tensor.matmul(
                        banks[w][0:W, 0:C],
                        asg[:, t * 2 * W : (t + 1) * 2 * W]
                            .rearrange("p (ko m) -> p ko m", ko=2),
                        xt[:, t * C2 : (t + 1) * C2]
                            .rearrange("p (ko c) -> p ko c", ko=2),
                        start=(iw == 0),
                        stop=(iw == half - 1),
                        perf_mode=mybir.MatmulPerfMode.DoubleRow,
                        tile_position=(0, 0),
                    )

            # ---- epilogue-only constants, behind the payload stream ----
            nc.scalar.dma_start(wb_t[:], wbase_b)
            nc.scalar.dma_start(v1_t[:], v1_b)
            nc.scalar.dma_start(m2_t[:], m2_b)
            nc.scalar.dma_start(v0_t[:], v0_b)
            nc.scalar.dma_start(w4_t[:], w4_b)
            nc.scalar.dma_start(bbt[:], bb_t)
            nc.scalar.dma_start(b4t[:], b4_t)
            nc.scalar.dma_start(rec_t[:], rec_b)

            # ---- epilogue (per-graph folded MLP), one pass per window ----
            # window w lives in partitions [0:32] of banks[w]; outputs are
            # assembled as columns of one [32, WPC] tile.
            oo = epool.tile([128, WPC], fp32, tag="oo")
            for w in range(WPC):
                ew = nc.vector
                rw = rec_t[0:W, w : w + 1]

                # t1 = (Sx * recip) * W_base ; gp0 = rowsum(t1)
                t1 = epool.tile([128, D], fp32, tag=f"t1{w}")
                gp = epool.tile([128, 1], fp32, tag=f"gp{w}")
                ew.scalar_tensor_tensor(
                    t1[0:W], banks[w][0:W, 0:D], rw, wb_t[0:W],
                    op0=mybir.AluOpType.mult, op1=mybir.AluOpType.mult,
                    accum_out=gp[0:W],
                )
                # mr = Sr * recip
                mr = epool.tile([128, R], fp32, tag=f"mr{w}")
                ew.tensor_scalar_mul(mr[0:W], banks[w][0:W, D : D + R], rw)
                # gp += b_base
                ew.tensor_add(gp[0:W], gp[0:W], bbt[0:W])
                # pre = gp*v1 + v0  (+ sum_j mr[:,j]*M2[j])
                pre = epool.tile([128, 32], fp32, tag=f"pre{w}")
                ew.scalar_tensor_tensor(
                    pre[0:W], v1_t[0:W], gp[0:W], v0_t[0:W],
                    op0=mybir.AluOpType.mult, op1=mybir.AluOpType.add,
                )
                for j in range(R):
                    ew.scalar_tensor_tensor(
                        pre[0:W], m2_t[0:W, j * 32 : (j + 1) * 32],
                        mr[0:W, j : j + 1], pre[0:W],
                        op0=mybir.AluOpType.mult, op1=mybir.AluOpType.add,
                    )
                act = epool.tile([128, 32], fp32, tag=f"act{w}")
                nc.scalar.activation(act[0:W], pre[0:W],
                                     mybir.ActivationFunctionType.Relu)
                # oo[:, w] = rowsum(act * W4) + b4
                prod = epool.tile([128, 32], fp32, tag=f"prod{w}")
                ew.tensor_mul(prod[0:W], act[0:W], w4_t[0:W])
                ew.tensor_reduce(oo[0:W, w : w + 1], prod[0:W],
                                 axis=mybir.AxisListType.X,
                                 op=mybir.AluOpType.add)
                ew.tensor_add(oo[0:W, w : w + 1], oo[0:W, w : w + 1], b4t[0:W])

            # out row w*32+p  <-  oo[p, w]
            nc.sync.dma_start(
                out_d.rearrange("(w p) o -> p (w o)", w=WPC), oo[0:W, :])

    nc.compile()
    return nc


def kernel(x, clock_period, batch, W_base, b_base, W1, b1, W2, b2, W3, b3, W4, b4,
           _profile=None):
    x = np.asarray(x, np.float32)
    clock = np.asarray(clock_period, np.float32).reshape(-1)
    batch = np.asarray(batch, np.int32)
    W_base = np.asarray(W_base, np.float32)
    W1 = np.asarray(W1, np.float32); b1 = np.asarray(b1, np.float32)
    W2 = np.asarray(W2, np.float32); b2 = np.asarray(b2, np.float32)
    W3 = np.asarray(W3, np.float32); b3 = np.asarray(b3, np.float32)
    W4 = np.asarray(W4, np.float32); b4 = np.asarray(b4, np.float32)
    hid = W1.shape[1]

    # r-path: exact algebraic fold when relu(c*W1 + b1) == c * relu(W1)
    fold = bool(np.all(b1 == 0.0)) and bool(clock.min() >= 0.0)
    if fold:
        R = 1
        r32 = clock[:, None]                                   # [N, 1]
        q = np.maximum(W1, 0.0) @ W2                           # [1, hid]
        M2 = q @ W3[1:, :]                                     # [1, 32]
        v0 = b2 @ W3[1:, :] + b3                               # [32]
    else:
        R = hid
        r32 = np.maximum(clock[:, None] @ W1 + b1, 0.0)        # [N, hid]
        M2 = W2 @ W3[1:, :]                                    # [hid, 32]
        v0 = b2 @ W3[1:, :] + b3

    C = D + R               # fp8 payload: [x | r]
    C2 = 2 * C

    # ---- shard by graph; window padding so tile->window map is static ----
    cut = np.searchsorted(batch, np.arange(0, N_GRAPHS + 1, W))
    win_nodes = np.diff(cut)
    T_w = int(math.ceil(win_nodes.max() / 128.0))
    # T_w even (DoubleRow pairs stay in-window); pick a super-tile size
    # ST | n_dr with ST in [36, 52] (~1.2-1.7 MB DMA transfers)
    def _pick_st(n_dr):
        for st in range(52, 35, -1):
            if n_dr % st == 0:
                return st
        return None
    T_w += T_w % 2
    while True:
        n_dr = WPC * T_w // 2
        ST = _pick_st(n_dr)
        if ST is not None:
            break
        T_w += 2
    n128 = WPC * T_w
    S = n_dr // ST
    Npad = n128 * 128

    x8 = x.astype(F8)
    r8 = r32.astype(F8)

    # per-graph counts (same host metadata as the shard layout)
    counts = np.diff(np.searchsorted(batch, np.arange(0, N_GRAPHS + 1)))
    recip = (1.0 / np.maximum(counts, 1)).astype(np.float32)

    in_maps = []
    # shared constant tiles
    iota_c = np.broadcast_to(
        np.tile(np.arange(W, dtype=F8), 2 * ST)[None, :], (128, 2 * ST * W)
    ).copy()
    wbase_b = np.broadcast_to(W_base[:, 0][None, :], (128, D)).astype(np.float32).copy()
    v1_b = np.broadcast_to(W3[0, :][None, :], (128, 32)).astype(np.float32).copy()
    m2_b = np.broadcast_to(M2.reshape(-1)[None, :], (128, R * 32)).astype(np.float32).copy()
    v0_b = np.broadcast_to(v0[None, :], (128, 32)).astype(np.float32).copy()
    w4_b = np.broadcast_to(W4[:, 0][None, :], (128, 32)).astype(np.float32).copy()
    bb_t = np.full((128, 1), float(b_base.reshape(-1)[0]), np.float32)
    b4_t = np.full((128, 1), float(b4.reshape(-1)[0]), np.float32)

    for k in range(N_CORES):
        xcc = np.zeros((Npad, C), F8)
        br = np.full(Npad, -1.0, F8)
        for wi in range(WPC):
            gw = k * WPC + wi          # global window index
            s0, e0 = int(cut[gw]), int(cut[gw + 1])
            n = e0 - s0
            o = wi * T_w * 128
            xcc[o : o + n, 0:D] = x8[s0:e0]
            xcc[o : o + n, D : D + R] = r8[s0:e0]
            br[o : o + n] = (batch[s0:e0] - gw * W).astype(F8)
        brs = np.ascontiguousarray(br.reshape(n128, 128).T)
        # DoubleRow packing: DR-tile d holds nodes [d*256, (d+1)*256), with
        # slot (p, ko) = node d*256 + ko*128 + p; each partition line is
        # contiguous in DRAM per super-tile.
        xcc_p = np.ascontiguousarray(
            xcc.reshape(S, ST, 2, 128, C).transpose(0, 3, 1, 2, 4)
        ).reshape(S, 128, ST * C2)
        # col w, partitions [0:32] = window w's graphs
        rk = recip[k * GPC : (k + 1) * GPC]
        rec_k = np.ones((128, WPC), np.float32)
        rec_k[0:W, :] = rk.reshape(WPC, W).T
        in_maps.append(dict(
            xcc=xcc_p, brs=brs, iota_c=iota_c,
            wbase_b=wbase_b, v1_b=v1_b, m2_b=m2_b, v0_b=v0_b, w4_b=w4_b,
            bb_t=bb_t, b4_t=b4_t, rec_b=rec_k,
        ))

    nc = _build_program(S, ST, C)

    kw = {}
    if _profile is not None:
        kw = dict(trace=True, **_profile)
    res = run_bass_kernel_spmd(nc, in_maps, list(range(N_CORES)), **kw)

    out = np.concatenate([res.results[k]["out"] for k in range(N_CORES)], axis=0)
    if _profile is not None:
        return out.astype(np.float32), res
    return out.astype(np.float32)
